# revision 18
# baseline (speedup 1.0000x reference)
"""Trainium2 Bass kernel for nn_MemoryUnit (scatter_memory).

Computes: att = softmax(x @ mem.T / 0.5); att = hard_shrink_relu(att, 0.005);
att = att / max(L1(att), eps); out = att @ mem.

Sharding: data-parallel over N across 8 cores; mem replicated per core.

Per 128-row tile (m = 2048 memory slots), with the softmax denominator and
any per-row positive scale cancelled by the L1 renormalization:
  logits = x16 @ mem16.T            (single fp16 matmul, fp32 PSUM accum)
  e = exp(2*logits)                 (f16; ACT, rowsums via ACT accumulators)
  mask = (e * (1/lam)) > S_e        (one DVE tensor_scalar pass, 4x mode)
  g = mask * e                      (one DVE tensor_tensor pass, 2x mode)
  out_ps = gT.T @ [mem | 1]         (PE; ones column gives L1(g) for free)
  out = out_ps[:, :128] / max(out_ps[:, 128], tiny)

Engine placement:
  - PE: mm1 + mm2 only (mm2 uses gT chunks as the stationary operand so the
    output is out[n, z] directly - no output transpose).
  - DMA XBAR (sync queue): x pre-transpose in 16-tile chunks, g transpose.
  - ACT: the two exp halves + their accumulator reads (rowsums).
  - DVE: rowsum add, mask, masked mult, clamp+reciprocal, final scale.
  - GPSIMD: f16 x chunk cast loads, output stores.

Every stage's inputs are produced >= 1 pipeline group earlier (distinct
skews), per-tag pools cover their skew span + slack, and emission is
oldest-tile-first so no engine queue ever head-of-line blocks on the
current group.
"""

import sys

sys.path.insert(0, "/opt/trn_rl_repo")

import numpy as np

N_FULL = 131072
Z = 128
M = 2048
P = 128
N_CORES = 8
LAM = 0.005
CHUNK = 16          # tiles per x pre-transpose chunk

_cache = {}


def _build(n_rows: int):
    import os
    import concourse.bass as bass
    import concourse.bacc as bacc
    import concourse.mybir as mybir
    import concourse.tile as tile

    LPS_BUFS = int(os.environ.get("K_LPS", "3"))
    OPS_BUFS = int(os.environ.get("K_OPS", "2"))
    FIN_ACT = os.environ.get("K_FINACT", "0") == "1"
    SKF = int(os.environ.get("K_SKF", "14"))
    SKM2 = int(os.environ.get("K_SKM2", "13"))
    STQ = os.environ.get("K_STQ", "sync")
    GTP = int(os.environ.get("K_GTP", "12"))
    EP = int(os.environ.get("K_EP", "9"))

    f32 = mybir.dt.float32
    f16 = mybir.dt.float16
    Alu = mybir.AluOpType
    Act = mybir.ActivationFunctionType

    NT = n_rows // P
    assert n_rows % P == 0 and NT % CHUNK == 0
    NC_CH = M // P      # 16 mem chunks
    HB = M // 2         # 1024: exp half width
    ZE = Z + 1          # mm2 moving width (mem | ones)

    nc = bacc.Bacc("TRN2", target_bir_lowering=False, debug=False, num_devices=1)
    x_d = nc.dram_tensor("x", [n_rows, Z], f32, kind="ExternalInput")
    mem_d = nc.dram_tensor("mem", [M, Z], f32, kind="ExternalInput")
    out_d = nc.dram_tensor("out", [n_rows, Z], f32, kind="ExternalOutput")

    with tile.TileContext(nc) as tc:
        with (
            tc.tile_pool(name="consts", bufs=1) as consts,
            tc.tile_pool(name="xcp", bufs=2) as xcp,
            tc.tile_pool(name="xtp", bufs=2) as xtp,
            tc.tile_pool(name="ep", bufs=EP) as ep,
            tc.tile_pool(name="mp", bufs=5) as mp,
            tc.tile_pool(name="gp", bufs=5) as gp,
            tc.tile_pool(name="gtp", bufs=GTP) as gtp,
            tc.tile_pool(name="s1p", bufs=6) as s1p,
            tc.tile_pool(name="sep", bufs=6) as sep,
            tc.tile_pool(name="rsp", bufs=4) as rsp,
            tc.tile_pool(name="outp", bufs=2) as outp,
            tc.tile_pool(name="lps", bufs=LPS_BUFS, space="PSUM") as lps,
            tc.tile_pool(name="ops", bufs=OPS_BUFS, space="PSUM") as ops,
        ):
            # ---------- preamble ----------
            # mem as f16 chunks + a ones column: mhE[p, c, 0:Z] = mem[c*128+p, :],
            # mhE[p, c, Z] = 1.0  (mm2 moving operand; ones col accumulates L1(g))
            mem_sb = consts.tile([P, NC_CH, Z], f32)
            nc.sync.dma_start(
                mem_sb[:], mem_d.ap().rearrange("(c p) z -> p c z", p=P)
            )
            mhE = consts.tile([P, NC_CH, ZE], f16)
            nc.vector.tensor_copy(out=mhE[:, :, 0:Z], in_=mem_sb[:])
            nc.vector.memset(mhE[:, :, Z:ZE], 1.0)
            # contiguous f16 copy of mem for the XBAR transpose source
            mh = consts.tile([P, NC_CH, Z], f16)
            nc.vector.tensor_copy(out=mh[:], in_=mem_sb[:])
            # mem^T via DMA XBAR: mhT[z, c, p] = mem[c*128+p, z] -> flat [z, m]
            mhT = consts.tile([P, NC_CH, P], f16)
            nc.sync.dma_start_transpose(mhT[:], mh[:])
            mhT_flat = mhT[:].rearrange("z c p -> z (c p)")

            # x: cast-load + XBAR-transpose per 16-tile chunk.
            # xT[c][z, j, n] = x[c*2048 + j*128 + n, z]
            # First chunks are prepped in the preamble; the rest are emitted
            # inside the pipeline (prefetched PREF chunks ahead of use).
            NCH = NT // CHUNK
            PREF = 3
            xT = {}
            xc_tiles = {}

            def load_chunk(c):
                r0 = c * CHUNK * P
                src_ap = x_d.ap()[r0:r0 + CHUNK * P, :].rearrange(
                    "(j p) z -> p j z", p=P
                )
                xc = xcp.tile([P, CHUNK, Z], f16, tag="xc", name="xc",
                              bufs=PREF + 1)
                nc.gpsimd.dma_start(xc[:], src_ap)
                xc_tiles[c] = xc

            def transpose_chunk(c):
                xT[c] = xtp.tile([P, CHUNK, P], f16, tag="xT", name="xT",
                                 bufs=PREF + 2)
                nc.sync.dma_start_transpose(xT[c][:], xc_tiles.pop(c)[:])

            for c in range(min(PREF, NCH)):
                load_chunk(c)
                transpose_chunk(c)

            # PE warm-up: a continuous matmul chain long enough to ramp the
            # PE p-state to max (>3us busy) before the pipeline starts.
            warm = lps.tile([P, HB], f32, tag="logits", name="warm")
            for k in range(14):
                nc.tensor.matmul(
                    warm[:, 0:512], mhT_flat[:, 0:P], mhT_flat[:, 0:512],
                    start=True, stop=True,
                )

            # ---------- pipeline state ----------
            st = [dict() for _ in range(NT)]

            def stage_mm1(i):
                # logits = xh @ mh^T (single fp16 limb, fp32 PSUM accum).
                # Prefetch the x chunk PREF chunks ahead: the load starts at
                # a chunk boundary; its XBAR transpose is issued 8 tiles
                # later so it never head-of-line blocks the sync queue
                # behind a still-running SWDGE load.
                if i % CHUNK == 0:
                    nxt = i // CHUNK + PREF
                    if nxt < NCH:
                        load_chunk(nxt)
                elif i % CHUNK == 8:
                    nxt = i // CHUNK + PREF
                    if nxt < NCH:
                        transpose_chunk(nxt)
                s = st[i]
                xst = xT[i // CHUNK][:, i % CHUNK, :]
                s["logits"] = [
                    lps.tile([P, HB], f32, tag="logits", name="logits")
                    for _ in range(2)
                ]
                for k in range(4):
                    h, b = divmod(k, 2)
                    ii = nc.tensor.matmul(
                        s["logits"][h][:, b * 512:(b + 1) * 512],
                        xst,
                        mhT_flat[:, k * 512:(k + 1) * 512],
                        start=True, stop=True,
                    )
                    if k != 0:
                        ii.ins.ldweights = False

            def stage_exp(i):
                s = st[i]
                s["e"] = ep.tile([P, M], f16, tag="e", name="e")
                s["s1h"] = s1p.tile([P, 2], f32, tag="s1h", name="s1h")
                nc.scalar.activation(
                    s["e"][:, 0:HB], s["logits"][0][:],
                    Act.Exp, scale=2.0,
                    accum_out=s["s1h"][:, 0:1],
                )
                nc.scalar.activation(
                    s["e"][:, HB:M], s["logits"][1][:],
                    Act.Exp, scale=2.0,
                    accum_out=s["s1h"][:, 1:2],
                )
                s.pop("logits")

            def stage_sum(i):
                # S_e = s1h[0] + s1h[1]
                s = st[i]
                s["Se"] = sep.tile([P, 1], f32, tag="Se", name="Se")
                nc.vector.tensor_tensor(
                    out=s["Se"][:], in0=s["s1h"][:, 0:1], in1=s["s1h"][:, 1:2],
                    op=Alu.add,
                )
                s.pop("s1h")

            def stage_mask(i):
                # mask = (e * (1/lam)) > S_e   (f16 0/1; TS runs in 4x mode)
                s = st[i]
                s["mk"] = mp.tile([P, M], f16, tag="mk", name="mk")
                nc.vector.tensor_scalar(
                    out=s["mk"][:], in0=s["e"][:],
                    scalar1=1.0 / LAM, scalar2=s["Se"][:],
                    op0=Alu.mult, op1=Alu.is_gt,
                )
                s.pop("Se")

            def stage_g(i):
                # g = mask * e   (f16; TT runs in 2x mode)
                s = st[i]
                s["g"] = gp.tile([P, M], f16, tag="g", name="g")
                nc.vector.tensor_tensor(
                    out=s["g"][:], in0=s["mk"][:], in1=s["e"][:],
                    op=Alu.mult,
                )
                s.pop("e")
                s.pop("mk")

            def stage_gt(i):
                # g [n, m] -> gT[p, c, n] = g[n, c*128+p] via DMA XBAR
                s = st[i]
                s["gT"] = gtp.tile([P, NC_CH, P], f16, tag="gT", name="gT")
                nc.sync.dma_start_transpose(s["gT"][:], s["g"][:])
                s.pop("g")

            def stage_mm2(i):
                # out[n, 0:Z] += gT_c^T @ mem_c ; out[n, Z] += sum_m g[n, m]
                s = st[i]
                out_ps = ops.tile([P, ZE], f32, tag="out_ps", name="out_ps")
                s["out_ps"] = out_ps
                for c in range(NC_CH):
                    nc.tensor.matmul(
                        out_ps[:], s["gT"][:, c, :], mhE[:, c, :],
                        start=(c == 0), stop=(c == NC_CH - 1),
                    )
                s.pop("gT")

            BST = 8     # tiles per batched output store
            fin_bufs = {}

            def stage_fin(i):
                s = st[i]
                Sc = rsp.tile([P, 1], f32, tag="Sc", name="Sc")
                nc.vector.tensor_scalar_max(Sc[:], s["out_ps"][:, Z:ZE], 1e-32)
                rS = rsp.tile([P, 1], f32, tag="rS", name="rS")
                nc.vector.reciprocal(rS[:], Sc[:])
                j = i % BST
                if j == 0:
                    fin_bufs[i // BST] = outp.tile([P, BST, Z], f32,
                                                   tag="fin", name="fin")
                fin = fin_bufs[i // BST]
                if FIN_ACT:
                    nc.scalar.activation(fin[:, j, :], s["out_ps"][:, 0:Z],
                                         Act.Copy, scale=rS[:])
                else:
                    nc.vector.tensor_scalar_mul(fin[:, j, :],
                                                s["out_ps"][:, 0:Z], rS[:])
                s.pop("out_ps")

            def stage_store(i):
                # store batch b when its last fin is several groups old, so
                # the store's wait never head-of-line blocks the sync queue
                j = i % BST
                if j != BST - 1 and i != NT - 1:
                    return
                b = i // BST
                fin = fin_bufs.pop(b)
                r0 = b * BST * P
                dst = out_d.ap()[r0:r0 + BST * P, :].rearrange(
                    "(j p) z -> p j z", p=P
                )
                if STQ == "sync":
                    nc.sync.dma_start(dst, fin[:])
                elif STQ == "act":
                    nc.scalar.dma_start(dst, fin[:])
                else:
                    nc.gpsimd.dma_start(dst, fin[:])

            # ---------- software-pipelined emission ----------
            # Distinct skews; every stage's inputs come from earlier groups.
            # Emission is oldest-tile-first (descending skew) within a group.
            SK_MM1, SK_EXP, SK_SUM, SK_MASK, SK_G, SK_GT = 2, 3, 4, 5, 6, 7
            SK_MM2, SK_FIN = SKM2, SKF
            SK_ST = SKF + 4
            LAST = SK_ST

            stages = [
                (SK_ST, stage_store),
                (SK_FIN, stage_fin),
                (SK_MM2, stage_mm2),
                (SK_GT, stage_gt),
                (SK_G, stage_g),
                (SK_MASK, stage_mask),
                (SK_SUM, stage_sum),
                (SK_EXP, stage_exp),
                (SK_MM1, stage_mm1),
            ]
            import os
            _last = os.environ.get("K_LAST")
            if _last:
                _names = {stage_mm1: "mm1", stage_mm2: "mm2", stage_exp: "exp",
                          stage_sum: "sum", stage_mask: "mask", stage_g: "g",
                          stage_gt: "gt", stage_fin: "fin", stage_store: "store"}
                _chain = ["mm1", "exp", "sum", "mask", "g", "gt", "mm2", "fin",
                          "store"]
                _keep = set(_chain[:_chain.index(_last) + 1])
                stages = [(sk, fn) for sk, fn in stages if _names[fn] in _keep]
            for s_idx in range(NT + LAST):
                for skew, fn in stages:
                    i = s_idx - skew
                    if 0 <= i < NT:
                        fn(i)

    nc.compile()
    return nc


def _get_nc(n_rows: int):
    if n_rows not in _cache:
        _cache[n_rows] = _build(n_rows)
    return _cache[n_rows]


def kernel(x: np.ndarray, mem: np.ndarray) -> np.ndarray:
    from concourse.bass_utils import run_bass_kernel_spmd

    x = np.ascontiguousarray(np.asarray(x, dtype=np.float32))
    mem = np.ascontiguousarray(np.asarray(mem, dtype=np.float32))
    n = x.shape[0]
    assert n % N_CORES == 0
    n_loc = n // N_CORES
    nc = _get_nc(n_loc)
    in_maps = [
        {"x": x[i * n_loc:(i + 1) * n_loc], "mem": mem} for i in range(N_CORES)
    ]
    # transient NRT/device errors happen occasionally; retry a couple times
    last_err = None
    for _ in range(3):
        try:
            res = run_bass_kernel_spmd(nc, in_maps, list(range(N_CORES)))
            break
        except Exception as err:  # noqa: BLE001
            last_err = err
            import time as _time
            _time.sleep(10)
    else:
        raise last_err
    out = np.concatenate([r["out"] for r in res.results], axis=0)
    return out.astype(np.float32)


# revision 19
# speedup vs baseline: 1.0986x; 1.0986x over previous
"""Trainium2 Bass kernel for nn_MemoryUnit (scatter_memory).

Computes: att = softmax(x @ mem.T / 0.5); att = hard_shrink_relu(att, 0.005);
att = att / max(L1(att), eps); out = att @ mem.

Sharding: data-parallel over N across 8 cores; mem replicated per core.

Per 128-row tile (m = 2048 memory slots), with the softmax denominator and
any per-row positive scale cancelled by the L1 renormalization:
  logits = x16 @ mem16.T            (single fp16 matmul, fp32 PSUM accum)
  e = exp(2*logits)                 (f16; ACT, rowsums via ACT accumulators)
  mask = (e * (1/lam)) > S_e        (one DVE tensor_scalar pass, 4x mode)
  g = mask * e                      (one DVE tensor_tensor pass, 2x mode)
  out_ps = gT.T @ [mem | 1]         (PE; ones column gives L1(g) for free)
  out = out_ps[:, :128] / max(out_ps[:, 128], tiny)

Engine placement:
  - PE: mm1 + mm2 only (mm2 uses gT chunks as the stationary operand so the
    output is out[n, z] directly - no output transpose).
  - DMA XBAR (sync queue): x pre-transpose in 16-tile chunks, g transpose.
  - ACT: the two exp halves + their accumulator reads (rowsums).
  - DVE: rowsum add, mask, masked mult, clamp+reciprocal, final scale.
  - GPSIMD: f16 x chunk cast loads, output stores.

Every stage's inputs are produced >= 1 pipeline group earlier (distinct
skews), per-tag pools cover their skew span + slack, and emission is
oldest-tile-first so no engine queue ever head-of-line blocks on the
current group.
"""

import sys

sys.path.insert(0, "/opt/trn_rl_repo")

import numpy as np

N_FULL = 131072
Z = 128
M = 2048
P = 128
N_CORES = 8
LAM = 0.005
CHUNK = 16          # tiles per x pre-transpose chunk

_cache = {}


def _build(n_rows: int):
    import os
    import concourse.bass as bass
    import concourse.bacc as bacc
    import concourse.mybir as mybir
    import concourse.tile as tile

    LPS_BUFS = int(os.environ.get("K_LPS", "3"))
    OPS_BUFS = int(os.environ.get("K_OPS", "2"))
    FIN_ACT = os.environ.get("K_FINACT", "0") == "1"
    SKF = int(os.environ.get("K_SKF", "14"))
    SKM2 = int(os.environ.get("K_SKM2", "13"))
    STQ = os.environ.get("K_STQ", "sync")
    GTP = int(os.environ.get("K_GTP", "10"))
    EP = int(os.environ.get("K_EP", "8"))

    f32 = mybir.dt.float32
    f16 = mybir.dt.float16
    Alu = mybir.AluOpType
    Act = mybir.ActivationFunctionType

    NT = n_rows // P
    assert n_rows % P == 0 and NT % CHUNK == 0
    NC_CH = M // P      # 16 mem chunks
    HB = M // 2         # 1024: exp half width
    ZE = Z + 1          # mm2 moving width (mem | ones)

    nc = bacc.Bacc("TRN2", target_bir_lowering=False, debug=False, num_devices=1)
    x_d = nc.dram_tensor("x", [n_rows, Z], f32, kind="ExternalInput")
    mem_d = nc.dram_tensor("mem", [M, Z], f32, kind="ExternalInput")
    out_d = nc.dram_tensor("out", [n_rows, Z], f32, kind="ExternalOutput")

    with tile.TileContext(nc) as tc:
        with (
            tc.tile_pool(name="consts", bufs=1) as consts,
            tc.tile_pool(name="xcp", bufs=2) as xcp,
            tc.tile_pool(name="xtp", bufs=2) as xtp,
            tc.tile_pool(name="ep", bufs=EP) as ep,
            tc.tile_pool(name="mp", bufs=5) as mp,
            tc.tile_pool(name="gp", bufs=5) as gp,
            tc.tile_pool(name="gtp", bufs=GTP) as gtp,
            tc.tile_pool(name="s1p", bufs=6) as s1p,
            tc.tile_pool(name="sep", bufs=6) as sep,
            tc.tile_pool(name="rsp", bufs=4) as rsp,
            tc.tile_pool(name="outp", bufs=2) as outp,
            tc.tile_pool(name="lps", bufs=LPS_BUFS, space="PSUM") as lps,
            tc.tile_pool(name="ops", bufs=OPS_BUFS, space="PSUM") as ops,
        ):
            # ---------- preamble ----------
            # mem as f16 chunks + a ones column: mhE[p, c, 0:Z] = mem[c*128+p, :],
            # mhE[p, c, Z] = 1.0  (mm2 moving operand; ones col accumulates L1(g))
            mem_sb = consts.tile([P, NC_CH, Z], f32)
            nc.sync.dma_start(
                mem_sb[:], mem_d.ap().rearrange("(c p) z -> p c z", p=P)
            )
            mhE = consts.tile([P, NC_CH, ZE], f16)
            nc.vector.tensor_copy(out=mhE[:, :, 0:Z], in_=mem_sb[:])
            nc.vector.memset(mhE[:, :, Z:ZE], 1.0)
            # contiguous f16 copy of mem for the XBAR transpose source
            mh = consts.tile([P, NC_CH, Z], f16)
            nc.vector.tensor_copy(out=mh[:], in_=mem_sb[:])
            # mem^T via DMA XBAR: mhT[z, c, p] = mem[c*128+p, z] -> flat [z, m]
            mhT = consts.tile([P, NC_CH, P], f16)
            nc.sync.dma_start_transpose(mhT[:], mh[:])
            mhT_flat = mhT[:].rearrange("z c p -> z (c p)")

            # x: cast-load + XBAR-transpose per 16-tile chunk.
            # xT[c][z, j, n] = x[c*2048 + j*128 + n, z]
            # First chunks are prepped in the preamble; the rest are emitted
            # inside the pipeline (prefetched PREF chunks ahead of use).
            NCH = NT // CHUNK
            PREF = 3
            xT = {}
            xc_tiles = {}

            def load_chunk(c):
                r0 = c * CHUNK * P
                src_ap = x_d.ap()[r0:r0 + CHUNK * P, :].rearrange(
                    "(j p) z -> p j z", p=P
                )
                xc = xcp.tile([P, CHUNK, Z], f16, tag="xc", name="xc",
                              bufs=PREF + 1)
                nc.gpsimd.dma_start(xc[:], src_ap)
                xc_tiles[c] = xc

            def transpose_chunk(c):
                xT[c] = xtp.tile([P, CHUNK, P], f16, tag="xT", name="xT",
                                 bufs=PREF + 2)
                nc.sync.dma_start_transpose(xT[c][:], xc_tiles.pop(c)[:])

            for c in range(min(PREF, NCH)):
                load_chunk(c)
                transpose_chunk(c)

            # PE warm-up: a continuous matmul chain long enough to ramp the
            # PE p-state to max (>3us busy) before the pipeline starts.
            warm = lps.tile([P, HB], f32, tag="logits", name="warm")
            for k in range(14):
                nc.tensor.matmul(
                    warm[:, 0:512], mhT_flat[:, 0:P], mhT_flat[:, 0:512],
                    start=True, stop=True,
                )

            # ---------- pipeline state ----------
            st = [dict() for _ in range(NT)]

            def stage_mm1(i):
                # logits = xh @ mh^T (single fp16 limb, fp32 PSUM accum).
                # Prefetch the x chunk PREF chunks ahead: the load starts at
                # a chunk boundary; its XBAR transpose is issued 8 tiles
                # later so it never head-of-line blocks the sync queue
                # behind a still-running SWDGE load.
                if i % CHUNK == 0:
                    nxt = i // CHUNK + PREF
                    if nxt < NCH:
                        load_chunk(nxt)
                elif i % CHUNK == 8:
                    nxt = i // CHUNK + PREF
                    if nxt < NCH:
                        transpose_chunk(nxt)
                s = st[i]
                xst = xT[i // CHUNK][:, i % CHUNK, :]
                s["logits"] = [
                    lps.tile([P, HB], f32, tag="logits", name="logits")
                    for _ in range(2)
                ]
                for k in range(4):
                    h, b = divmod(k, 2)
                    ii = nc.tensor.matmul(
                        s["logits"][h][:, b * 512:(b + 1) * 512],
                        xst,
                        mhT_flat[:, k * 512:(k + 1) * 512],
                        start=True, stop=True,
                    )
                    if k != 0:
                        ii.ins.ldweights = False

            def stage_exp(i):
                s = st[i]
                s["e"] = ep.tile([P, M], f16, tag="e", name="e")
                s["s1h"] = s1p.tile([P, 2], f32, tag="s1h", name="s1h")
                nc.scalar.activation(
                    s["e"][:, 0:HB], s["logits"][0][:],
                    Act.Exp, scale=2.0,
                    accum_out=s["s1h"][:, 0:1],
                )
                nc.scalar.activation(
                    s["e"][:, HB:M], s["logits"][1][:],
                    Act.Exp, scale=2.0,
                    accum_out=s["s1h"][:, 1:2],
                )
                s.pop("logits")

            def stage_sum(i):
                # S_e = s1h[0] + s1h[1]
                s = st[i]
                s["Se"] = sep.tile([P, 1], f32, tag="Se", name="Se")
                nc.vector.tensor_tensor(
                    out=s["Se"][:], in0=s["s1h"][:, 0:1], in1=s["s1h"][:, 1:2],
                    op=Alu.add,
                )
                s.pop("s1h")

            def stage_mask(i):
                # mask = (e * (1/lam)) > S_e   (f16 0/1; TS runs in 4x mode)
                s = st[i]
                s["mk"] = mp.tile([P, M], f16, tag="mk", name="mk")
                nc.vector.tensor_scalar(
                    out=s["mk"][:], in0=s["e"][:],
                    scalar1=1.0 / LAM, scalar2=s["Se"][:],
                    op0=Alu.mult, op1=Alu.is_gt,
                )
                s.pop("Se")

            def stage_g(i):
                # g = mask * e   (f16; TT runs in 2x mode)
                s = st[i]
                s["g"] = gp.tile([P, M], f16, tag="g", name="g")
                nc.vector.tensor_tensor(
                    out=s["g"][:], in0=s["mk"][:], in1=s["e"][:],
                    op=Alu.mult,
                )
                s.pop("e")
                s.pop("mk")

            def stage_gt(i):
                # g [n, m] -> gT[p, c, n] = g[n, c*128+p] via DMA XBAR
                s = st[i]
                s["gT"] = gtp.tile([P, NC_CH, P], f16, tag="gT", name="gT")
                nc.sync.dma_start_transpose(s["gT"][:], s["g"][:])
                s.pop("g")

            def stage_mm2(i):
                # out[n, 0:Z] += gT_c^T @ mem_c ; out[n, Z] += sum_m g[n, m]
                s = st[i]
                out_ps = ops.tile([P, ZE], f32, tag="out_ps", name="out_ps")
                s["out_ps"] = out_ps
                for c in range(NC_CH):
                    nc.tensor.matmul(
                        out_ps[:], s["gT"][:, c, :], mhE[:, c, :],
                        start=(c == 0), stop=(c == NC_CH - 1),
                    )
                s.pop("gT")

            BST = 8     # tiles per batched output store
            fin_bufs = {}

            def stage_fin(i):
                s = st[i]
                Sc = rsp.tile([P, 1], f32, tag="Sc", name="Sc")
                nc.vector.tensor_scalar_max(Sc[:], s["out_ps"][:, Z:ZE], 1e-32)
                rS = rsp.tile([P, 1], f32, tag="rS", name="rS")
                nc.vector.reciprocal(rS[:], Sc[:])
                j = i % BST
                if j == 0:
                    fin_bufs[i // BST] = outp.tile([P, BST, Z], f32,
                                                   tag="fin", name="fin")
                fin = fin_bufs[i // BST]
                if FIN_ACT:
                    nc.scalar.activation(fin[:, j, :], s["out_ps"][:, 0:Z],
                                         Act.Copy, scale=rS[:])
                else:
                    nc.vector.tensor_scalar_mul(fin[:, j, :],
                                                s["out_ps"][:, 0:Z], rS[:])
                s.pop("out_ps")

            def stage_store(i):
                # store batch b when its last fin is several groups old, so
                # the store's wait never head-of-line blocks the sync queue
                j = i % BST
                if j != BST - 1 and i != NT - 1:
                    return
                b = i // BST
                fin = fin_bufs.pop(b)
                r0 = b * BST * P
                dst = out_d.ap()[r0:r0 + BST * P, :].rearrange(
                    "(j p) z -> p j z", p=P
                )
                if STQ == "sync":
                    nc.sync.dma_start(dst, fin[:])
                elif STQ == "act":
                    nc.scalar.dma_start(dst, fin[:])
                else:
                    nc.gpsimd.dma_start(dst, fin[:])

            # ---------- software-pipelined emission ----------
            # Distinct skews; every stage's inputs come from earlier groups.
            # Emission is oldest-tile-first (descending skew) within a group.
            SK_MM1, SK_EXP, SK_SUM, SK_MASK, SK_G, SK_GT = 2, 3, 4, 5, 6, 7
            SK_MM2, SK_FIN = SKM2, SKF
            SK_ST = SKF + 4
            LAST = SK_ST

            stages = [
                (SK_ST, stage_store),
                (SK_FIN, stage_fin),
                (SK_MM2, stage_mm2),
                (SK_GT, stage_gt),
                (SK_G, stage_g),
                (SK_MASK, stage_mask),
                (SK_SUM, stage_sum),
                (SK_EXP, stage_exp),
                (SK_MM1, stage_mm1),
            ]
            import os
            _last = os.environ.get("K_LAST")
            if _last:
                _names = {stage_mm1: "mm1", stage_mm2: "mm2", stage_exp: "exp",
                          stage_sum: "sum", stage_mask: "mask", stage_g: "g",
                          stage_gt: "gt", stage_fin: "fin", stage_store: "store"}
                _chain = ["mm1", "exp", "sum", "mask", "g", "gt", "mm2", "fin",
                          "store"]
                _keep = set(_chain[:_chain.index(_last) + 1])
                stages = [(sk, fn) for sk, fn in stages if _names[fn] in _keep]
            for s_idx in range(NT + LAST):
                for skew, fn in stages:
                    i = s_idx - skew
                    if 0 <= i < NT:
                        fn(i)

    nc.compile()
    return nc


def _get_nc(n_rows: int):
    if n_rows not in _cache:
        _cache[n_rows] = _build(n_rows)
    return _cache[n_rows]


def kernel(x: np.ndarray, mem: np.ndarray) -> np.ndarray:
    from concourse.bass_utils import run_bass_kernel_spmd

    x = np.ascontiguousarray(np.asarray(x, dtype=np.float32))
    mem = np.ascontiguousarray(np.asarray(mem, dtype=np.float32))
    n = x.shape[0]
    assert n % N_CORES == 0
    n_loc = n // N_CORES
    nc = _get_nc(n_loc)
    in_maps = [
        {"x": x[i * n_loc:(i + 1) * n_loc], "mem": mem} for i in range(N_CORES)
    ]
    # transient NRT/device errors happen occasionally; retry a couple times
    last_err = None
    for _ in range(3):
        try:
            res = run_bass_kernel_spmd(nc, in_maps, list(range(N_CORES)))
            break
        except Exception as err:  # noqa: BLE001
            last_err = err
            import time as _time
            _time.sleep(10)
    else:
        raise last_err
    out = np.concatenate([r["out"] for r in res.results], axis=0)
    return out.astype(np.float32)
